# revision 19
# baseline (speedup 1.0000x reference)
"""AttentionWithFastKANTransform — Trainium2 Bass kernel, 8-core SPMD.

Sharding: core c handles batch b=c//4, query rows [qc*512,(qc+1)*512), qc=c%4.
Each core computes the full FastKAN K/V projections for its batch (duplicated
across the 4 cores sharing a batch), its own query/gate projections, attention
for its query block, and the final FastKAN — zero cross-core communication.

RBF basis exp(-((xn-g_j)/den)^2) is one ACT op per grid point via
Derivative_Erf (= 2/sqrt(pi)*exp(-x^2)); the 2/sqrt(pi) and the 1/sqrt(HD)
attention scale are folded into the spline weights host-side. Spline/base
matmuls run in fp16, attention scores in fp32r (scores are bounded ~[-25,25]
so softmax needs no max-subtraction), probabilities in bf16 (fp16 would
underflow the softmax sums). Softmax denominators come free from a ones-column
appended to wv. Silu is the native ACT function; the gate sigmoid is
0.5*tanh(0.5x)+0.5 (tanh shares silu's table set) — no DVE reciprocals on the
hot path (reciprocal runs at 8 cycles/element). ACT instructions are chained
into 7 table-set phases (ln_exp -> silu -> erf -> exp -> ln_exp -> silu ->
erf) via explicit dependency edges so table loads stay at 7 total.
"""
import math
import numpy as np

import concourse.bass as bass
import concourse.bacc as bacc
import concourse.tile as tile
from concourse import mybir
from concourse.bass_utils import run_bass_kernel_spmd
from concourse.masks import make_identity
from concourse.tile_rust import add_dep_helper

B, LQ, LK = 2, 2048, 2048
D = 256
H, HD = 8, 32
NG = 8
NCORES = 8
QR = 512            # query rows per core
LN_EPS = 1e-6
F32 = mybir.dt.float32
F32R = mybir.dt.float32r
F16 = mybir.dt.float16
BF16 = mybir.dt.bfloat16
AF = mybir.ActivationFunctionType
OP = mybir.AluOpType
P = 128


def _build(nc, grid, den):
    from contextlib import ExitStack
    inv_den = 1.0 / den
    with tile.TileContext(nc) as tc, ExitStack() as stack:
        # ---------------- DRAM I/O ----------------
        xq_d = nc.dram_tensor("xq", [QR, D], F32, kind="ExternalInput").ap()
        xk_d = nc.dram_tensor("xk", [LK, D], F32, kind="ExternalInput").ap()
        xv_d = nc.dram_tensor("xv", [LK, D], F32, kind="ExternalInput").ap()
        wsp_d, wb_d, bb_d = {}, {}, {}
        for pn in ("q", "k", "v", "g", "o"):
            wsp_d[pn] = nc.dram_tensor(f"wsp_{pn}", [D * NG, D], F16, kind="ExternalInput").ap()
            wb_d[pn] = nc.dram_tensor(f"wb_{pn}", [D, D], F16, kind="ExternalInput").ap()
            bb_d[pn] = nc.dram_tensor(f"bb_{pn}", [D], F32, kind="ExternalInput").ap()
        out_d = nc.dram_tensor("out", [QR, D], F32, kind="ExternalOutput").ap()

        # ---------------- persistent SBUF ----------------
        const = stack.enter_context(tc.tile_pool(name="const", bufs=1))
        wpool = stack.enter_context(tc.tile_pool(name="wpool", bufs=1))
        big = stack.enter_context(tc.tile_pool(name="big", bufs=1))

        ident = const.tile([P, P], F32)
        make_identity(nc, ident)
        eps_t = const.tile([P, 1], F32)
        nc.vector.memset(eps_t, LN_EPS)
        one_t = const.tile([P, 1], F32)
        nc.vector.memset(one_t, 1.0)
        h05 = const.tile([P, 1], F32)
        nc.vector.memset(h05, 0.5)
        gbias = const.tile([P, NG], F32)
        for j in range(NG):
            nc.vector.memset(gbias[:, j:j + 1], -grid[j] * inv_den)

        wsp_s, wb_s, bbp_s, bbb_s = {}, {}, {}, {}

        def load_weights(pn):
            wsp_s[pn] = wpool.tile([P, 16, D], F16, tag="wsp_q" if pn == "o" else f"wsp_{pn}", name=f"wsp_s_{pn}")
            nc.sync.dma_start(out=wsp_s[pn], in_=wsp_d[pn].rearrange("(kc p) o -> p kc o", p=P))
            wb_s[pn] = wpool.tile([P, 2, D], F16, tag="wb_q" if pn == "o" else f"wb_{pn}", name=f"wb_s_{pn}")
            nc.sync.dma_start(out=wb_s[pn], in_=wb_d[pn].rearrange("(fc p) o -> p fc o", p=P))
            bbp_s[pn] = wpool.tile([P, 2], F32, tag=f"bbp_{pn}", name=f"bbp_s_{pn}")
            nc.sync.dma_start(out=bbp_s[pn], in_=bb_d[pn].rearrange("(c p) -> p c", p=P))
            bbb_s[pn] = wpool.tile([P, D], F32, tag=f"bbb_{pn}", name=f"bbb_s_{pn}")
            nc.sync.dma_start(out=bbb_s[pn], in_=bass.AP(
                tensor=bb_d[pn].tensor, offset=bb_d[pn].offset, ap=[[0, P], [1, D]]))

        for pn in ("q", "k", "v", "g"):
            load_weights(pn)

        xnT, silT = {}, {}
        for pn, R in (("q", QR), ("k", LK), ("v", LK)):
            xnT[pn] = big.tile([P, 2, R], F32, tag=f"xnT_{pn}", name=f"xnT_{pn}")
            silT[pn] = big.tile([P, 2, R], F16, tag=f"silT_{pn}", name=f"silT_{pn}")
        wqT = big.tile([P, 2, QR], F32R)
        wkT = big.tile([P, 2, LK], F32R)
        wv_aug = big.tile([P, 16, H * (HD + 1)], BF16)
        glog = big.tile([P, 4, D], F32)
        gate = big.tile([P, 4, D], F32)
        o_sb = big.tile([P, 4, D], F32)
        nc.vector.memset(
            wv_aug.rearrange("p kc (h c) -> p (kc h) c", c=HD + 1)[:, :, HD:HD + 1], 1.0)

        acts = [[] for _ in range(7)]  # per-phase ACT instructions (table ordering)

        # ============ PASS A: LN + transpose + silu for q, k, v ============
        def ln_transpose_silu(x_src, pn, R, pool, psp, rlist, slist, src_sb=None):
            nch = R // P
            for g0 in range(0, nch, 4):
                gn = min(4, nch - g0)
                mvs = pool.tile([P, 4, 2], F32, tag="mvs")
                xss = []
                for i in range(gn):
                    r = g0 + i
                    if src_sb is not None:
                        xs = src_sb[:, r, :]
                    else:
                        xs = pool.tile([P, D], F32, tag=f"xs{i}", name=f"xs{i}")
                        nc.sync.dma_start(out=xs, in_=x_src[r * P:(r + 1) * P, :])
                    xss.append(xs)
                    stats = pool.tile([P, 6], F32, tag="st")
                    nc.vector.bn_stats(out=stats, in_=xs)
                    nc.vector.bn_aggr(out=mvs[:, i, :], in_=stats)
                rstd = pool.tile([P, 4], F32, tag="rs")
                rlist.append(nc.scalar.activation(rstd[:, 0:gn], mvs[:, 0:gn, 1], AF.Ln, bias=eps_t))
                rlist.append(nc.scalar.activation(rstd[:, 0:gn], rstd[:, 0:gn], AF.Exp, scale=-0.5))
                for i in range(gn):
                    r = g0 + i
                    xn = pool.tile([P, D], F32, tag="xn")
                    nc.vector.tensor_scalar(xn, xss[i], mvs[:, i, 0:1], rstd[:, i:i + 1],
                                            op0=OP.subtract, op1=OP.mult)
                    for fc in range(2):
                        tp = psp.tile([P, P], F32, tag="tp")
                        nc.tensor.transpose(tp, xn[:, fc * P:(fc + 1) * P], ident)
                        nc.vector.tensor_copy(xnT[pn][:, fc, r * P:(r + 1) * P], tp)
            # native Silu ACT, 1024-col chunks
            for cc in range(R // 512):
                xt = xnT[pn][:, :, cc * 512:(cc + 1) * 512]
                slist.append(nc.scalar.activation(
                    silT[pn][:, :, cc * 512:(cc + 1) * 512], xt, AF.Silu))

        with tc.tile_pool(name="a_pool", bufs=3) as a_pool, \
             tc.tile_pool(name="a_ps", bufs=4, space="PSUM") as a_ps:
            ln_transpose_silu(xq_d, "q", QR, a_pool, a_ps, acts[0], acts[1])
            ln_transpose_silu(xk_d, "k", LK, a_pool, a_ps, acts[0], acts[1])
            ln_transpose_silu(xv_d, "v", LK, a_pool, a_ps, acts[0], acts[1])

        # shared big scratch: basis tiles (B, D) and expT (C)
        scratch = stack.enter_context(tc.tile_pool(name="scratch", bufs=2))

        def basis_group(pn, g, alist):
            bt = scratch.tile([P, 16, 512], F16, tag="big16")
            btv = bt.rearrange("p (j fc) c -> p j fc c", fc=2)
            xt = xnT[pn][:, :, g * 512:(g + 1) * 512]
            for j in range(NG):
                alist.append(nc.scalar.activation(
                    btv[:, j, :, :], xt, AF.Derivative_Erf,
                    scale=inv_den, bias=gbias[:, j:j + 1]))
            return bt

        def proj_T(pn, dst, R, psp, alist, bts=None):
            for g in range(R // 512):
                bt = bts[g] if bts else basis_group(pn, g, alist)
                for m in range(2):
                    ps = psp.tile([P, 512], F32, tag="ps")
                    for kc in range(16):
                        nc.tensor.matmul(ps, wsp_s[pn][:, kc, m * P:(m + 1) * P],
                                         bt[:, kc, :], start=(kc == 0), stop=False)
                    for fc in range(2):
                        nc.tensor.matmul(ps, wb_s[pn][:, fc, m * P:(m + 1) * P],
                                         silT[pn][:, fc, g * 512:(g + 1) * 512],
                                         start=False, stop=(fc == 1))
                    nc.vector.tensor_scalar(
                        dst[:, m, g * 512:(g + 1) * 512], ps,
                        bbp_s[pn][:, m:m + 1], None, op0=OP.add)

        def proj_R(pn, xpn, evict_fn, R, psp, alist, bts=None):
            for g in range(R // 512):
                bt = bts[g] if bts else basis_group(xpn, g, alist)
                for rr in range(4):
                    ps = psp.tile([P, 512], F32, tag="ps")
                    psn = ps[:, 0:D]
                    for kc in range(16):
                        nc.tensor.matmul(psn, bt[:, kc, rr * P:(rr + 1) * P],
                                         wsp_s[pn][:, kc, :], start=(kc == 0), stop=False)
                    for fc in range(2):
                        nc.tensor.matmul(
                            psn,
                            silT[xpn][:, fc, g * 512 + rr * P:g * 512 + (rr + 1) * P],
                            wb_s[pn][:, fc, :], start=False, stop=(fc == 1))
                    evict_fn(psn, g * 4 + rr)

        # ============ PASS B: bases + all four projections =================
        with tc.tile_pool(name="b_ps", bufs=4, space="PSUM") as b_ps:
            bt_q = basis_group("q", 0, acts[2])
            proj_T("q", wqT, QR, b_ps, acts[2], bts=[bt_q])
            proj_R("g", "q",
                   lambda ps, r: nc.vector.tensor_add(glog[:, r, :], ps, bbb_s["g"]),
                   QR, b_ps, acts[2], bts=[bt_q])
            proj_T("k", wkT, LK, b_ps, acts[2])

            def evict_v(ps, r):
                dst = wv_aug[:, r, :].rearrange("p (h c) -> p h c", c=HD + 1)[:, :, 0:HD]
                nc.vector.tensor_add(dst, ps.rearrange("p (h c) -> p h c", c=HD),
                                     bbb_s["v"].rearrange("p (h c) -> p h c", c=HD))
            proj_R("v", "v", evict_v, LK, b_ps, acts[2])

        # ============ PASS C: attention + gate + o-LN/silu =================
        xnT["o"] = big.tile([P, 2, QR], F32, tag="xnT_q", name="xnT_o")
        silT["o"] = big.tile([P, 2, QR], F16, tag="silT_q", name="silT_o")
        with tc.tile_pool(name="c_pool", bufs=3) as c_pool, \
             tc.tile_pool(name="c_ps", bufs=2, space="PSUM") as c_ps, \
             tc.tile_pool(name="c_ps2", bufs=2, space="PSUM") as c_ps2, \
             tc.tile_pool(name="c_ps3", bufs=2, space="PSUM") as c_ps3:
            # gate = sigmoid(glog) = 0.5*tanh(0.5*glog) + 0.5
            t_g = c_pool.tile([P, 4, D], F32, tag="t_g")
            acts[3].append(nc.scalar.activation(t_g, glog, AF.Tanh, scale=0.5))
            nc.vector.tensor_scalar(gate, t_g, h05, h05, op0=OP.mult, op1=OP.add)

            for h in range(H):
                m, poff = h // 4, (h % 4) * 32
                tpos = (poff, 0) if poff == 96 else None
                expT = scratch.tile([P, 16, 512], BF16, tag="big16")
                for kc2 in range(8):
                    sc = c_ps.tile([P, 2, 512], F32, tag="sc")
                    for u in range(2):
                        kc = kc2 * 2 + u
                        nc.tensor.matmul(sc[:, u, :],
                                         wkT[poff:poff + 32, m, kc * P:(kc + 1) * P],
                                         wqT[poff:poff + 32, m, :],
                                         start=True, stop=True, tile_position=tpos)
                    acts[3].append(nc.scalar.activation(
                        expT[:, kc2 * 2:kc2 * 2 + 2, :], sc, AF.Exp))
                av = c_ps2.tile([HD + 1, 512], F32, tag="av")
                for kc in range(16):
                    nc.tensor.matmul(av, wv_aug[:, kc, h * (HD + 1):(h + 1) * (HD + 1)],
                                     expT[:, kc, :], start=(kc == 0), stop=(kc == 15))
                av_sb = c_pool.tile([HD + 1, 512], F32, tag="av_sb")
                nc.vector.tensor_copy(av_sb, av)
                for r in range(4):
                    tp = c_ps3.tile([P, HD + 1], F32, tag="tp")
                    nc.tensor.transpose(tp, av_sb[:, r * P:(r + 1) * P],
                                        ident[0:HD + 1, 0:HD + 1])
                    rinv = c_pool.tile([P, 1], F32, tag="rinv")
                    nc.vector.reciprocal(rinv, tp[:, HD:HD + 1])
                    onorm = c_pool.tile([P, HD], F32, tag="onorm")
                    nc.vector.tensor_scalar(onorm, tp[:, 0:HD], rinv, None, op0=OP.mult)
                    nc.vector.tensor_mul(o_sb[:, r, h * HD:(h + 1) * HD], onorm,
                                         gate[:, r, h * HD:(h + 1) * HD])

            ln_transpose_silu(None, "o", QR, c_pool, c_ps3, acts[4], acts[5], src_sb=o_sb)

        # ============ PASS D: final fastkan ================================
        load_weights("o")
        with tc.tile_pool(name="d_pool", bufs=3) as d_pool, \
             tc.tile_pool(name="d_ps", bufs=4, space="PSUM") as d_ps:
            def evict_out(ps, r):
                os_ = d_pool.tile([P, D], F32, tag="out_s")
                nc.vector.tensor_add(os_, ps, bbb_s["o"])
                nc.sync.dma_start(out=out_d[r * P:(r + 1) * P, :], in_=os_)
            proj_R("o", "o", evict_out, QR, d_ps, acts[6])

        for i in range(len(acts) - 1):
            if acts[i] and acts[i + 1]:
                add_dep_helper(acts[i + 1][0].ins, acts[i][-1].ins,
                               reason="ACT table-set phase ordering")
    return nc


_CACHE = {}


def _get_nc(grid, den):
    key = (tuple(grid), den)
    if key not in _CACHE:
        nc = bacc.Bacc(None, target_bir_lowering=False, debug=False)
        _build(nc, grid, den)
        nc.compile()
        _CACHE[key] = nc
    return _CACHE[key]


def _in_maps(q, k, v, params):
    norm = HD ** (-0.5)
    cbase = math.sqrt(math.pi) / 2.0  # undo the 2/sqrt(pi) of Derivative_Erf
    wmaps = {}
    for pn, p in params.items():
        ln_s, ln_b, grid_p, den_p, spline_w, base_w, base_b = [np.asarray(x) for x in p]
        assert np.all(ln_s == 1.0) and np.all(ln_b == 0.0), "non-trivial LN affine"
        s = cbase * (norm if pn == "q" else 1.0)
        wsp = spline_w.reshape(D, D, NG).transpose(2, 1, 0).reshape(D * NG, D) * s
        sb = norm if pn == "q" else 1.0
        wmaps[f"wsp_{pn}"] = np.ascontiguousarray(wsp).astype(np.float16)
        wmaps[f"wb_{pn}"] = np.ascontiguousarray(base_w * sb).astype(np.float16)
        wmaps[f"bb_{pn}"] = np.ascontiguousarray(base_b * sb).astype(np.float32)
    maps = []
    for c in range(NCORES):
        b, qc = c // 4, c % 4
        maps.append({
            "xq": np.ascontiguousarray(q[b, qc * QR:(qc + 1) * QR]),
            "xk": np.ascontiguousarray(k[b]),
            "xv": np.ascontiguousarray(v[b]),
            **wmaps,
        })
    return maps


def _run(q, k, v, p_q, p_k, p_v, p_g, p_o, trace=False):
    q = np.asarray(q, np.float32)
    k = np.asarray(k, np.float32)
    v = np.asarray(v, np.float32)
    params = {"q": p_q, "k": p_k, "v": p_v, "g": p_g, "o": p_o}
    grid = [float(x) for x in np.asarray(params["q"][2])]
    den = float(np.asarray(params["q"][3]))
    nc = _get_nc(grid, den)
    maps = _in_maps(q, k, v, params)
    res = run_bass_kernel_spmd(nc, maps, list(range(NCORES)), trace=trace)
    out = np.empty((B, LQ, D), np.float32)
    for c in range(NCORES):
        b, qc = c // 4, c % 4
        out[b, qc * QR:(qc + 1) * QR] = res.results[c]["out"]
    return out, res


def kernel(q, k, v, p_q, p_k, p_v, p_g, p_o):
    out, _ = _run(q, k, v, p_q, p_k, p_v, p_g, p_o)
    return out


def run_traced(inputs):
    out, res = _run(inputs["q"], inputs["k"], inputs["v"], inputs["p_q"],
                    inputs["p_k"], inputs["p_v"], inputs["p_g"], inputs["p_o"],
                    trace=True)
    res.out = out
    return res


# revision 26
# speedup vs baseline: 1.1683x; 1.1683x over previous
"""AttentionWithFastKANTransform — Trainium2 Bass kernel, 8-core SPMD.

Sharding: core c handles batch b=c//4, query rows [qc*512,(qc+1)*512), qc=c%4.
Each core computes the full FastKAN K/V projections for its batch (duplicated
across the 4 cores sharing a batch), its own query/gate projections, attention
for its query block, and the final FastKAN — zero cross-core communication.

RBF basis exp(-((xn-g_j)/den)^2) is one ACT op per grid point via
Derivative_Erf (= 2/sqrt(pi)*exp(-x^2)); the 2/sqrt(pi) and the 1/sqrt(HD)
attention scale are folded into the spline weights host-side. Spline/base
matmuls run in fp16, attention scores in fp32r (scores are bounded ~[-25,25]
so softmax needs no max-subtraction), probabilities in bf16 (fp16 would
underflow the softmax sums). Softmax denominators come free from a ones-column
appended to wv. Silu is the native ACT function; the gate sigmoid is
0.5*tanh(0.5x)+0.5 (tanh shares silu's table set) — no DVE reciprocals on the
hot path (reciprocal runs at 8 cycles/element). ACT instructions are chained
into 7 table-set phases (ln_exp -> silu -> erf -> exp -> ln_exp -> silu ->
erf) via explicit dependency edges so table loads stay at 7 total.

Known headroom (next session): de-duplicate the K/V projections via an 8-rank
AllGather. Note: 4-rank replica groups [[0..3],[4..7]] are rejected by bass
("shared output not supported for 4 cores (needs >4)"), so the gather must be
a single 8-rank group with batch-interleaved shards — either split attention
per batch (q spans 256 per batch) or mask-combine the two batch halves with
per-core 0/1 input masks after the gather. Projected saving ~50us ACT +
~55us PE of the duplicated basis/spline work.
"""
import math
import numpy as np

import concourse.bass as bass
import concourse.bacc as bacc
import concourse.tile as tile
from concourse import mybir
from concourse.bass_utils import run_bass_kernel_spmd
from concourse.masks import make_identity
from concourse.tile_rust import add_dep_helper

B, LQ, LK = 2, 2048, 2048
D = 256
H, HD = 8, 32
NG = 8
NCORES = 8
QR = 512            # query rows per core
LN_EPS = 1e-6
F32 = mybir.dt.float32
F32R = mybir.dt.float32r
F16 = mybir.dt.float16
BF16 = mybir.dt.bfloat16
AF = mybir.ActivationFunctionType
OP = mybir.AluOpType
P = 128


def _build(nc, grid, den):
    from contextlib import ExitStack
    inv_den = 1.0 / den
    with tile.TileContext(nc) as tc, ExitStack() as stack:
        # ---------------- DRAM I/O ----------------
        xq_d = nc.dram_tensor("xq", [QR, D], F32, kind="ExternalInput").ap()
        xk_d = nc.dram_tensor("xk", [LK, D], F32, kind="ExternalInput").ap()
        xv_d = nc.dram_tensor("xv", [LK, D], F32, kind="ExternalInput").ap()
        wsp_d, wb_d, bb_d = {}, {}, {}
        for pn in ("q", "k", "v", "g", "o"):
            wsp_d[pn] = nc.dram_tensor(f"wsp_{pn}", [D * NG, D], F16, kind="ExternalInput").ap()
            wb_d[pn] = nc.dram_tensor(f"wb_{pn}", [D, D], F16, kind="ExternalInput").ap()
            bb_d[pn] = nc.dram_tensor(f"bb_{pn}", [D], F32, kind="ExternalInput").ap()
        out_d = nc.dram_tensor("out", [QR, D], F32, kind="ExternalOutput").ap()

        # ---------------- persistent SBUF ----------------
        const = stack.enter_context(tc.tile_pool(name="const", bufs=1))
        wpool = stack.enter_context(tc.tile_pool(name="wpool", bufs=1))
        big = stack.enter_context(tc.tile_pool(name="big", bufs=1))

        ident = const.tile([P, P], F32)
        make_identity(nc, ident)
        eps_t = const.tile([P, 1], F32)
        nc.vector.memset(eps_t, LN_EPS)
        one_t = const.tile([P, 1], F32)
        nc.vector.memset(one_t, 1.0)
        h05 = const.tile([P, 1], F32)
        nc.vector.memset(h05, 0.5)
        gbias = const.tile([P, NG], F32)
        for j in range(NG):
            nc.vector.memset(gbias[:, j:j + 1], -grid[j] * inv_den)

        wsp_s, wb_s, bbp_s, bbb_s = {}, {}, {}, {}

        def load_weights(pn):
            wsp_s[pn] = wpool.tile([P, 16, D], F16, tag="wsp_q" if pn == "o" else f"wsp_{pn}", name=f"wsp_s_{pn}")
            nc.sync.dma_start(out=wsp_s[pn], in_=wsp_d[pn].rearrange("(kc p) o -> p kc o", p=P))
            wb_s[pn] = wpool.tile([P, 2, D], F16, tag="wb_q" if pn == "o" else f"wb_{pn}", name=f"wb_s_{pn}")
            nc.sync.dma_start(out=wb_s[pn], in_=wb_d[pn].rearrange("(fc p) o -> p fc o", p=P))
            bbp_s[pn] = wpool.tile([P, 2], F32, tag=f"bbp_{pn}", name=f"bbp_s_{pn}")
            nc.sync.dma_start(out=bbp_s[pn], in_=bb_d[pn].rearrange("(c p) -> p c", p=P))
            bbb_s[pn] = wpool.tile([P, D], F32, tag=f"bbb_{pn}", name=f"bbb_s_{pn}")
            nc.sync.dma_start(out=bbb_s[pn], in_=bass.AP(
                tensor=bb_d[pn].tensor, offset=bb_d[pn].offset, ap=[[0, P], [1, D]]))

        xnT, silT = {}, {}
        for pn, R in (("q", QR), ("k", LK), ("v", LK)):
            xnT[pn] = big.tile([P, 2, R], F32, tag=f"xnT_{pn}", name=f"xnT_{pn}")
            silT[pn] = big.tile([P, 2, R], F16, tag=f"silT_{pn}", name=f"silT_{pn}")
        wqT = big.tile([P, 2, QR], F32R)
        wkT = big.tile([P, 2, LK], F32R)
        wv_aug = big.tile([P, 16, H * (HD + 1)], BF16)
        glog = big.tile([P, 4, D], F32)
        gate = big.tile([P, 4, D], F32)
        o_sb = big.tile([P, 4, D], F32)
        nc.vector.memset(
            wv_aug.rearrange("p kc (h c) -> p (kc h) c", c=HD + 1)[:, :, HD:HD + 1], 1.0)

        acts = [[] for _ in range(7)]  # per-phase ACT instructions (table ordering)

        # ============ PASS A: LN + transpose + silu for q, k, v ============
        def ln_transpose_silu(x_src, pn, R, pool, psp, rlist, slist, src_sb=None):
            nch = R // P
            for g0 in range(0, nch, 4):
                gn = min(4, nch - g0)
                mvs = pool.tile([P, 4, 2], F32, tag="mvs")
                xss = []
                for i in range(gn):
                    r = g0 + i
                    if src_sb is not None:
                        xs = src_sb[:, r, :]
                    else:
                        xs = pool.tile([P, D], F32, tag=f"xs{i}", name=f"xs{i}")
                        nc.sync.dma_start(out=xs, in_=x_src[r * P:(r + 1) * P, :])
                    xss.append(xs)
                    stats = pool.tile([P, 6], F32, tag="st")
                    nc.vector.bn_stats(out=stats, in_=xs)
                    nc.vector.bn_aggr(out=mvs[:, i, :], in_=stats)
                rstd = pool.tile([P, 4], F32, tag="rs")
                rlist.append(nc.scalar.activation(rstd[:, 0:gn], mvs[:, 0:gn, 1], AF.Ln, bias=eps_t))
                rlist.append(nc.scalar.activation(rstd[:, 0:gn], rstd[:, 0:gn], AF.Exp, scale=-0.5))
                for i in range(gn):
                    r = g0 + i
                    xn = pool.tile([P, D], F32, tag="xn")
                    nc.vector.tensor_scalar(xn, xss[i], mvs[:, i, 0:1], rstd[:, i:i + 1],
                                            op0=OP.subtract, op1=OP.mult)
                    for fc in range(2):
                        tp = psp.tile([P, P], F32, tag="tp")
                        nc.tensor.transpose(tp, xn[:, fc * P:(fc + 1) * P], ident)
                        if fc == 0:
                            nc.vector.tensor_copy(xnT[pn][:, fc, r * P:(r + 1) * P], tp)
                        else:
                            nc.scalar.copy(xnT[pn][:, fc, r * P:(r + 1) * P], tp)
            # native Silu ACT, 1024-col chunks
            for cc in range(R // 512):
                xt = xnT[pn][:, :, cc * 512:(cc + 1) * 512]
                slist.append(nc.scalar.activation(
                    silT[pn][:, :, cc * 512:(cc + 1) * 512], xt, AF.Silu))

        with tc.tile_pool(name="a_pool", bufs=3) as a_pool, \
             tc.tile_pool(name="a_ps", bufs=4, space="PSUM") as a_ps:
            ln_transpose_silu(xq_d, "q", QR, a_pool, a_ps, acts[0], acts[1])
            ln_transpose_silu(xk_d, "k", LK, a_pool, a_ps, acts[0], acts[1])
            ln_transpose_silu(xv_d, "v", LK, a_pool, a_ps, acts[0], acts[1])

        for pn in ("q", "k", "v", "g"):
            load_weights(pn)

        # shared big scratch: basis tiles (B, D) and expT (C)
        scratch = stack.enter_context(tc.tile_pool(name="scratch", bufs=2))

        def basis_group(pn, g, alist):
            bt = scratch.tile([P, 16, 512], F16, tag="big16")
            btv = bt.rearrange("p (j fc) c -> p j fc c", fc=2)
            xt = xnT[pn][:, :, g * 512:(g + 1) * 512]
            for j in range(NG):
                alist.append(nc.scalar.activation(
                    btv[:, j, :, :], xt, AF.Derivative_Erf,
                    scale=inv_den, bias=gbias[:, j:j + 1]))
            return bt

        def proj_T(pn, dst, R, psp, alist, bts=None):
            for g in range(R // 512):
                bt = bts[g] if bts else basis_group(pn, g, alist)
                for m in range(2):
                    ps = psp.tile([P, 512], F32, tag="ps")
                    for kc in range(16):
                        nc.tensor.matmul(ps, wsp_s[pn][:, kc, m * P:(m + 1) * P],
                                         bt[:, kc, :], start=(kc == 0), stop=False)
                    for fc in range(2):
                        nc.tensor.matmul(ps, wb_s[pn][:, fc, m * P:(m + 1) * P],
                                         silT[pn][:, fc, g * 512:(g + 1) * 512],
                                         start=False, stop=(fc == 1))
                    nc.vector.tensor_scalar(
                        dst[:, m, g * 512:(g + 1) * 512], ps,
                        bbp_s[pn][:, m:m + 1], None, op0=OP.add)

        def proj_R(pn, xpn, evict_fn, R, psp, alist, bts=None):
            for g in range(R // 512):
                bt = bts[g] if bts else basis_group(xpn, g, alist)
                for rr in range(4):
                    ps = psp.tile([P, 512], F32, tag="ps")
                    psn = ps[:, 0:D]
                    for kc in range(16):
                        nc.tensor.matmul(psn, bt[:, kc, rr * P:(rr + 1) * P],
                                         wsp_s[pn][:, kc, :], start=(kc == 0), stop=False)
                    for fc in range(2):
                        nc.tensor.matmul(
                            psn,
                            silT[xpn][:, fc, g * 512 + rr * P:g * 512 + (rr + 1) * P],
                            wb_s[pn][:, fc, :], start=False, stop=(fc == 1))
                    evict_fn(psn, g * 4 + rr)

        # ============ PASS B: bases + all four projections =================
        with tc.tile_pool(name="b_ps", bufs=4, space="PSUM") as b_ps:
            bt_q = basis_group("q", 0, acts[2])
            proj_T("q", wqT, QR, b_ps, acts[2], bts=[bt_q])
            proj_R("g", "q",
                   lambda ps, r: nc.vector.tensor_add(glog[:, r, :], ps, bbb_s["g"]),
                   QR, b_ps, acts[2], bts=[bt_q])
            proj_T("k", wkT, LK, b_ps, acts[2])

            def evict_v(ps, r):
                dst = wv_aug[:, r, :].rearrange("p (h c) -> p h c", c=HD + 1)[:, :, 0:HD]
                nc.vector.tensor_add(dst, ps.rearrange("p (h c) -> p h c", c=HD),
                                     bbb_s["v"].rearrange("p (h c) -> p h c", c=HD))
            proj_R("v", "v", evict_v, LK, b_ps, acts[2])

        # ============ PASS C: attention + gate + o-LN/silu =================
        xnT["o"] = big.tile([P, 2, QR], F32, tag="xnT_q", name="xnT_o")
        silT["o"] = big.tile([P, 2, QR], F16, tag="silT_q", name="silT_o")
        with tc.tile_pool(name="c_pool", bufs=3) as c_pool, \
             tc.tile_pool(name="c_ps", bufs=2, space="PSUM") as c_ps, \
             tc.tile_pool(name="c_ps2", bufs=2, space="PSUM") as c_ps2, \
             tc.tile_pool(name="c_ps3", bufs=2, space="PSUM") as c_ps3:
            # gate = sigmoid(glog) = 0.5*tanh(0.5*glog) + 0.5
            t_g = c_pool.tile([P, 4, D], F32, tag="t_g")
            acts[3].append(nc.scalar.activation(t_g, glog, AF.Tanh, scale=0.5))
            nc.vector.tensor_scalar(gate, t_g, h05, h05, op0=OP.mult, op1=OP.add)

            for h in range(H):
                m, poff = h // 4, (h % 4) * 32
                tpos = (poff, 0) if poff == 96 else None
                expT = scratch.tile([P, 16, 512], BF16, tag="big16")
                for kc2 in range(8):
                    sc = c_ps.tile([P, 2, 512], F32, tag="sc")
                    for u in range(2):
                        kc = kc2 * 2 + u
                        nc.tensor.matmul(sc[:, u, :],
                                         wkT[poff:poff + 32, m, kc * P:(kc + 1) * P],
                                         wqT[poff:poff + 32, m, :],
                                         start=True, stop=True, tile_position=tpos)
                    acts[3].append(nc.scalar.activation(
                        expT[:, kc2 * 2:kc2 * 2 + 2, :], sc, AF.Exp))
                av = c_ps2.tile([HD + 1, 512], F32, tag="av")
                for kc in range(16):
                    nc.tensor.matmul(av, wv_aug[:, kc, h * (HD + 1):(h + 1) * (HD + 1)],
                                     expT[:, kc, :], start=(kc == 0), stop=(kc == 15))
                av_sb = c_pool.tile([HD + 1, 512], F32, tag="av_sb")
                nc.vector.tensor_copy(av_sb, av)
                for r in range(4):
                    tp = c_ps3.tile([P, HD + 1], F32, tag="tp")
                    nc.tensor.transpose(tp, av_sb[:, r * P:(r + 1) * P],
                                        ident[0:HD + 1, 0:HD + 1])
                    rinv = c_pool.tile([P, 1], F32, tag="rinv")
                    nc.vector.reciprocal(rinv, tp[:, HD:HD + 1])
                    onorm = c_pool.tile([P, HD], F32, tag="onorm")
                    nc.vector.tensor_scalar(onorm, tp[:, 0:HD], rinv, None, op0=OP.mult)
                    nc.vector.tensor_mul(o_sb[:, r, h * HD:(h + 1) * HD], onorm,
                                         gate[:, r, h * HD:(h + 1) * HD])

            ln_transpose_silu(None, "o", QR, c_pool, c_ps3, acts[4], acts[5], src_sb=o_sb)

        # ============ PASS D: final fastkan ================================
        load_weights("o")
        with tc.tile_pool(name="d_pool", bufs=3) as d_pool, \
             tc.tile_pool(name="d_ps", bufs=4, space="PSUM") as d_ps:
            def evict_out(ps, r):
                os_ = d_pool.tile([P, D], F32, tag="out_s")
                nc.vector.tensor_add(os_, ps, bbb_s["o"])
                nc.sync.dma_start(out=out_d[r * P:(r + 1) * P, :], in_=os_)
            proj_R("o", "o", evict_out, QR, d_ps, acts[6])

        for i in range(len(acts) - 1):
            if acts[i] and acts[i + 1]:
                add_dep_helper(acts[i + 1][0].ins, acts[i][-1].ins,
                               reason="ACT table-set phase ordering")
    return nc


_CACHE = {}


def _get_nc(grid, den):
    key = (tuple(grid), den)
    if key not in _CACHE:
        nc = bacc.Bacc(None, target_bir_lowering=False, debug=False)
        _build(nc, grid, den)
        nc.compile()
        _CACHE[key] = nc
    return _CACHE[key]


def _in_maps(q, k, v, params):
    norm = HD ** (-0.5)
    cbase = math.sqrt(math.pi) / 2.0  # undo the 2/sqrt(pi) of Derivative_Erf
    wmaps = {}
    for pn, p in params.items():
        ln_s, ln_b, grid_p, den_p, spline_w, base_w, base_b = [np.asarray(x) for x in p]
        assert np.all(ln_s == 1.0) and np.all(ln_b == 0.0), "non-trivial LN affine"
        s = cbase * (norm if pn == "q" else 1.0)
        wsp = spline_w.reshape(D, D, NG).transpose(2, 1, 0).reshape(D * NG, D) * s
        sb = norm if pn == "q" else 1.0
        wmaps[f"wsp_{pn}"] = np.ascontiguousarray(wsp).astype(np.float16)
        wmaps[f"wb_{pn}"] = np.ascontiguousarray(base_w * sb).astype(np.float16)
        wmaps[f"bb_{pn}"] = np.ascontiguousarray(base_b * sb).astype(np.float32)
    maps = []
    for c in range(NCORES):
        b, qc = c // 4, c % 4
        maps.append({
            "xq": np.ascontiguousarray(q[b, qc * QR:(qc + 1) * QR]),
            "xk": np.ascontiguousarray(k[b]),
            "xv": np.ascontiguousarray(v[b]),
            **wmaps,
        })
    return maps


def _run(q, k, v, p_q, p_k, p_v, p_g, p_o, trace=False):
    q = np.asarray(q, np.float32)
    k = np.asarray(k, np.float32)
    v = np.asarray(v, np.float32)
    params = {"q": p_q, "k": p_k, "v": p_v, "g": p_g, "o": p_o}
    grid = [float(x) for x in np.asarray(params["q"][2])]
    den = float(np.asarray(params["q"][3]))
    nc = _get_nc(grid, den)
    maps = _in_maps(q, k, v, params)
    try:
        results = _run_cached_exec(nc, maps)
        res = None
    except Exception:
        res = run_bass_kernel_spmd(nc, maps, list(range(NCORES)), trace=trace)
        results = res.results
    out = np.empty((B, LQ, D), np.float32)
    for c in range(NCORES):
        b, qc = c // 4, c % 4
        out[b, qc * QR:(qc + 1) * QR] = results[c]["out"]
    return out, res


_EXEC = {}


def _run_cached_exec(nc, in_maps):
    """run_bass_via_pjrt with the shard_map-jitted executable cached across
    calls (bass2jax rebuilds and re-jits it per call, ~seconds of host time)."""
    import jax
    from jax.sharding import Mesh, PartitionSpec
    from jax.experimental.shard_map import shard_map
    from concourse import bass2jax, mybir as mb
    key = id(nc)
    if key not in _EXEC:
        bass2jax.install_neuronx_cc_hook()
        assert nc.dbg_addr is None
        partition_name = nc.partition_id_tensor.name if nc.partition_id_tensor else None
        in_names, out_names, out_avals, zero_outs = [], [], [], []
        for alloc in nc.m.functions[0].allocations:
            if not isinstance(alloc, mb.MemoryLocationSet):
                continue
            name = alloc.memorylocations[0].name
            if alloc.kind == "ExternalInput":
                if name != partition_name:
                    in_names.append(name)
            elif alloc.kind == "ExternalOutput":
                shape = tuple(alloc.tensor_shape)
                dtype = mb.dt.np(alloc.dtype)
                out_names.append(name)
                out_avals.append(jax.core.ShapedArray(shape, dtype))
                zero_outs.append(np.zeros(shape, dtype))
        n_params = len(in_names)
        all_names = list(in_names) + list(out_names)
        if partition_name is not None:
            all_names.append(partition_name)

        def _body(*args):
            operands = list(args)
            if partition_name is not None:
                operands.append(bass2jax.partition_id_tensor())
            return tuple(bass2jax._bass_exec_p.bind(
                *operands, out_avals=tuple(out_avals), in_names=tuple(all_names),
                out_names=tuple(out_names), lowering_input_output_aliases=(),
                sim_require_finite=True, sim_require_nnan=True, nc=nc))

        devices = jax.devices()[:NCORES]
        mesh = Mesh(np.asarray(devices), ("core",))
        n_io = n_params + len(out_names)
        sharded = jax.jit(
            shard_map(_body, mesh=mesh, in_specs=(PartitionSpec("core"),) * n_io,
                      out_specs=(PartitionSpec("core"),) * len(out_names),
                      check_rep=False),
            donate_argnums=tuple(range(n_params, n_io)), keep_unused=True)
        _EXEC[key] = (sharded, in_names, out_names, out_avals, zero_outs, n_params)
    sharded, in_names, out_names, out_avals, zero_outs, n_params = _EXEC[key]
    per_core = [[np.asarray(m[nm]) for nm in in_names] for m in in_maps]
    concat_in = [np.concatenate([per_core[c][i] for c in range(NCORES)], axis=0)
                 for i in range(n_params)]
    concat_zeros = [np.zeros((NCORES * z.shape[0], *z.shape[1:]), z.dtype)
                    for z in zero_outs]
    out_arrs = sharded(*concat_in, *concat_zeros)
    return [{name: np.asarray(out_arrs[i]).reshape(NCORES, *out_avals[i].shape)[c]
             for i, name in enumerate(out_names)} for c in range(NCORES)]


def kernel(q, k, v, p_q, p_k, p_v, p_g, p_o):
    out, _ = _run(q, k, v, p_q, p_k, p_v, p_g, p_o)
    return out


def run_traced(inputs):
    out, res = _run(inputs["q"], inputs["k"], inputs["v"], inputs["p_q"],
                    inputs["p_k"], inputs["p_v"], inputs["p_g"], inputs["p_o"],
                    trace=True)
    res.out = out
    return res
